# revision 10
# baseline (speedup 1.0000x reference)
"""Trainium2 kernel for nn_LocalPatternExtractor (binary-weight depthwise+pointwise
conv -> BatchNorm -> quantized LIF over 4 timesteps).

Forward-pass analysis
---------------------
The reference quantizes the membrane potential with
    step = THRESHOLD / 2**(POT_BITS-1) = 1/128
    q    = clip(round(v/step), -128, 127) * step
so after quantization  mem <= 127/128 = 0.9921875 < THRESHOLD (=1.0), with
f32 STE round-off bounded by ~|v|*2^-24 << 1/128.  Hence `mem >= THRESHOLD`
is false for every element at every timestep, no spike ever fires, and the
forward output is identically
    out      = zeros((B, C_out, L), float32)
    reg_loss = SPIKE_REG * mean(out) = 0.0
for *all* finite inputs (verified empirically against the jax reference for
several seeds and 10x-scaled inputs).  The optimal kernel therefore reduces
to materializing the zero output at HBM write roofline.

Sharding: pure data parallel over the batch dim (16 -> 2 per core on 8
cores); each core zero-fills its own (2, 256, 5000) f32 output shard
(10.24 MB), which the host concatenates.

Performance notes (from neuron-profile traces on the 8-core fleet):
- A core's 16 SDMA engines sustain ~25.4 GB/s each (~406 GB/s/core) when
  both HWDGE queues (sync + scalar) keep descriptors pending.
- DMA descriptors are assigned to the 16 SDMA engines round-robin by row
  order within each dma_start; only 128-row DMAs keep engine k aligned to
  its own SBUF port group (partitions 8k..8k+7).  A 112-row DMA (measured)
  still spreads over all 16 engines but misaligns rows to ports and drops
  the whole core to ~320 GB/s — so every chunk here spans all 128
  partitions.
- The DMA source is a small zero tile that every chunk re-reads, keeping
  the DVE memset off the critical path: a tiny tile A (fast to clear)
  feeds the first chunks while the bigger tile B is still being cleared.
"""

import numpy as np

import concourse.bass as bass
import concourse.mybir as mybir
from concourse.bass_utils import run_bass_kernel_spmd

N_CORES = 8
B, C_IN, L = 16, 12, 5000
C_OUT = 256

B_LOC = B // N_CORES               # 2 batches per core
OUT_ELEMS = B_LOC * C_OUT * L      # 2,560,000 f32 per core (10.24 MB)
P = 128                            # SBUF partitions
COLS = OUT_ELEMS // P              # 20,000 f32 per partition row

WA = 625          # tile A cols (2.5 KB per partition row)
WB = 2500         # tile B cols (10 KB per partition row)
# gpsimd clears tile A and immediately issues the first two chunks itself
# (same-engine ordering, no cross-engine semaphore hop), so packets start
# flowing as early as possible; sync and scalar each add one more tile-A
# chunk, then stream the seven tile-B chunks between them.
N_A = 4           # 4 chunks of WA cover [0, 2500): 2 gpsimd, 1 sync, 1 scalar
N_B = 7           # 7 chunks of WB cover [2500, 20000): 4 sync, 3 scalar
assert N_A * WA + N_B * WB == COLS

_cache: dict = {}


def _build() -> bass.Bass:
    nc = bass.Bass()
    out = nc.declare_dram_parameter("out", (P, COLS), mybir.dt.float32, isOutput=True)

    n_dma = N_A + N_B

    def chunk_a(i):  # col range of small chunk i
        return out[:, i * WA : (i + 1) * WA]

    def chunk_b(i):  # col range of big chunk i
        s = N_A * WA + i * WB
        return out[:, s : s + WB]

    with (
        nc.sbuf_tensor([P, WA], mybir.dt.float32) as zta,
        nc.sbuf_tensor([P, WB], mybir.dt.float32) as ztb,
        nc.semaphore("gsem") as gsem,
        nc.semaphore("msem") as msem,
        nc.semaphore("gdsem") as gdsem,
        nc.semaphore("dsem") as dsem,
        nc.Block() as block,
    ):

        @block.gpsimd
        def _(gpsimd):
            gpsimd.memset(zta[:], 0.0).then_inc(gsem, 1)
            # DMA dispatch is async w.r.t. the engine's compute pipeline, so
            # even same-engine consumers must wait on the memset's semaphore.
            gpsimd.wait_ge(gsem, 1)
            gpsimd.dma_start(chunk_a(0), zta[:]).then_inc(gdsem, 16)
            gpsimd.dma_start(chunk_a(1), zta[:]).then_inc(gdsem, 16)

        @block.vector
        def _(vector):
            vector.memset(ztb[:], 0.0).then_inc(msem, 1)

        @block.sync
        def _(sync):
            sync.wait_ge(gsem, 1)
            sync.dma_start(chunk_a(2), zta[:]).then_inc(dsem, 16)
            sync.wait_ge(msem, 1)
            for i in range(0, N_B, 2):
                sync.dma_start(chunk_b(i), ztb[:]).then_inc(dsem, 16)
            sync.wait_ge(dsem, 16 * (n_dma - 2))
            sync.wait_ge(gdsem, 32)

        @block.scalar
        def _(scalar):
            scalar.wait_ge(gsem, 1)
            scalar.dma_start(chunk_a(3), zta[:]).then_inc(dsem, 16)
            scalar.wait_ge(msem, 1)
            for i in range(1, N_B, 2):
                scalar.dma_start(chunk_b(i), ztb[:]).then_inc(dsem, 16)

    return nc


def get_nc() -> bass.Bass:
    nc = _cache.get("nc")
    if nc is None:
        nc = _cache["nc"] = _build()
    return nc


def kernel(x, dw_weight, pw_weight, gamma, beta):
    assert x.shape == (B, C_IN, L), x.shape
    nc = get_nc()
    res = run_bass_kernel_spmd(
        nc, [dict() for _ in range(N_CORES)], core_ids=list(range(N_CORES))
    )
    shards = [r["out"].reshape(B_LOC, C_OUT, L) for r in res.results]
    out = np.ascontiguousarray(np.concatenate(shards, axis=0))
    reg_loss = np.float32(0.01) * np.float32(out.mean(dtype=np.float64))
    return out, reg_loss
